# revision 4
# baseline (speedup 1.0000x reference)
"""Bit2Num dequantization kernel for Trainium2 (8 NeuronCores, SPMD).

Reference op: x [1024, 65536] of {0.0, 1.0} f32, B=4.
  bits = x.reshape(1024, 16384, 4)
  out[b, n] = (8*bits[b,n,0] + 4*bits[b,n,1] + 2*bits[b,n,2] + bits[b,n,3] + 0.5) / 16

Sharding: pure data-parallel over batch — 128 rows per core (= 128 SBUF
partitions). Per core: 32 MB in + 8 MB out => DMA-roofline-bound (~117 us
at ~358 GB/s HBM-per-NC).

Per-core kernel: pipeline over 8 column tiles of [128, 8192]:
  DMA-in tile -> 3 fused scalar_tensor_tensor ops on DVE (Horner combine of
  the 4 strided bit slices) -> final affine on ACT -> DMA-out [128, 2048].
Compute (~55 us DVE + ~8 us ACT) hides under DMA via tile double/triple
buffering.
"""

import numpy as np

import concourse.bacc as bacc
import concourse.bass as bass
import concourse.mybir as mybir
from concourse.bass_utils import run_bass_kernel_spmd
from concourse.tile import TileContext

N_CORES = 8
BATCH = 1024
COLS = 65536
B_BITS = 4
ROWS = BATCH // N_CORES          # 128 rows per core == SBUF partition count
OUT_COLS = COLS // B_BITS        # 16384
TILE_C = 8192                    # input cols per tile (32 KB / partition)
TILE_G = TILE_C // B_BITS        # 2048 output cols per tile
N_TILES = COLS // TILE_C         # 8

F32 = mybir.dt.float32
MULT = mybir.AluOpType.mult
ADD = mybir.AluOpType.add


def _build_nc() -> bass.Bass:
    # Bacc (not plain Bass): its compile() pipeline runs
    # generate_event_semaphores, which splits multi-wait sync conditions —
    # TRN2 DMA instructions accept at most one wait.
    nc = bacc.Bacc(None, target_bir_lowering=False)
    x = nc.dram_tensor("x", [ROWS, COLS], F32, kind="ExternalInput")
    out = nc.dram_tensor("out", [ROWS, OUT_COLS], F32, kind="ExternalOutput")

    with TileContext(nc) as tc:
        with (
            tc.tile_pool(name="xin", bufs=3) as xpool,
            tc.tile_pool(name="work", bufs=2) as wpool,
            tc.tile_pool(name="oout", bufs=3) as opool,
        ):
            for t in range(N_TILES):
                xt = xpool.tile([ROWS, TILE_C], F32, tag="xt")
                nc.sync.dma_start(
                    out=xt[:, :], in_=x[:, t * TILE_C:(t + 1) * TILE_C]
                )
                xv = xt[:, :].rearrange("p (g k) -> p g k", k=B_BITS)
                a = xv[:, :, 0]
                b = xv[:, :, 1]
                c = xv[:, :, 2]
                d = xv[:, :, 3]

                u = wpool.tile([ROWS, TILE_G], F32, tag="u")
                v = wpool.tile([ROWS, TILE_G], F32, tag="v")
                w = wpool.tile([ROWS, TILE_G], F32, tag="w")
                ot = opool.tile([ROWS, TILE_G], F32, tag="ot")

                # u = 2a + b ; v = 2c + d ; w = 4u + v = 8a+4b+2c+d
                nc.vector.scalar_tensor_tensor(
                    out=u[:, :], in0=a, scalar=2.0, in1=b, op0=MULT, op1=ADD
                )
                nc.vector.scalar_tensor_tensor(
                    out=v[:, :], in0=c, scalar=2.0, in1=d, op0=MULT, op1=ADD
                )
                nc.vector.scalar_tensor_tensor(
                    out=w[:, :], in0=u[:, :], scalar=4.0, in1=v[:, :],
                    op0=MULT, op1=ADD,
                )
                # ot = (w + 0.5) / 16 = w/16 + 1/32
                nc.scalar.activation(
                    out=ot[:, :], in_=w[:, :],
                    func=mybir.ActivationFunctionType.Copy,
                    bias=1.0 / 32.0, scale=1.0 / 16.0,
                )
                nc.sync.dma_start(
                    out=out[:, t * TILE_G:(t + 1) * TILE_G], in_=ot[:, :]
                )
    # Bacc.finalize runs the compile pipeline (register allocation +
    # generate_event_semaphores); the pjrt exec path serializes nc.m as-is.
    nc.finalize()
    return nc


_NC = None


def _get_nc() -> bass.Bass:
    global _NC
    if _NC is None:
        _NC = _build_nc()
    return _NC


def kernel(x: np.ndarray, B=4) -> np.ndarray:
    assert int(B) == B_BITS, f"kernel hardcodes B={B_BITS}, got {B}"
    x = np.ascontiguousarray(x, dtype=np.float32)
    assert x.shape == (BATCH, COLS), x.shape
    nc = _get_nc()
    in_maps = [{"x": x[i * ROWS:(i + 1) * ROWS]} for i in range(N_CORES)]
    res = run_bass_kernel_spmd(nc, in_maps, list(range(N_CORES)))
    return np.concatenate(
        [res.results[i]["out"] for i in range(N_CORES)], axis=0
    )


# revision 6
# speedup vs baseline: 1.0236x; 1.0236x over previous
"""Bit2Num dequantization kernel for Trainium2 (8 NeuronCores, SPMD).

Reference op: x [1024, 65536] of {0.0, 1.0} f32, B=4.
  bits = x.reshape(1024, 16384, 4)
  out[b, n] = (8*bits[b,n,0] + 4*bits[b,n,1] + 2*bits[b,n,2] + bits[b,n,3] + 0.5) / 16

Sharding: pure data-parallel over batch — 128 rows per core (= 128 SBUF
partitions). Per core: 32 MB in + 8 MB out => DMA-roofline-bound (~117 us
at ~358 GB/s HBM-per-NC).

Per-core kernel: pipeline over 8 column tiles of [128, 8192]:
  DMA-in tile -> 3 fused scalar_tensor_tensor ops on DVE (Horner combine of
  the 4 strided bit slices) -> final affine on ACT -> DMA-out [128, 2048].
Compute (~55 us DVE + ~8 us ACT) hides under DMA via tile double/triple
buffering.
"""

import numpy as np

import concourse.bacc as bacc
import concourse.bass as bass
import concourse.mybir as mybir
from concourse.bass_utils import run_bass_kernel_spmd
from concourse.tile import TileContext

N_CORES = 8
BATCH = 1024
COLS = 65536
B_BITS = 4
ROWS = BATCH // N_CORES          # 128 rows per core == SBUF partition count
OUT_COLS = COLS // B_BITS        # 16384
TILE_C = 8192                    # input cols per tile (32 KB / partition)
TILE_G = TILE_C // B_BITS        # 2048 output cols per tile
N_TILES = COLS // TILE_C         # 8

F32 = mybir.dt.float32
MULT = mybir.AluOpType.mult
ADD = mybir.AluOpType.add


def _build_nc() -> bass.Bass:
    # Bacc (not plain Bass): its compile() pipeline runs
    # generate_event_semaphores, which splits multi-wait sync conditions —
    # TRN2 DMA instructions accept at most one wait.
    nc = bacc.Bacc(None, target_bir_lowering=False)
    x = nc.dram_tensor("x", [ROWS, COLS], F32, kind="ExternalInput")
    out = nc.dram_tensor("out", [ROWS, OUT_COLS], F32, kind="ExternalOutput")

    # Compute/store in half-tiles: shrinks the end-of-kernel exposed tail
    # (last chunk's compute + store) and lets out-DMAs start earlier.
    HALF_C = TILE_C // 2             # 4096 input cols per compute chunk
    HALF_G = HALF_C // B_BITS        # 1024 output cols

    with TileContext(nc) as tc:
        with (
            tc.tile_pool(name="xin", bufs=3) as xpool,
            tc.tile_pool(name="work", bufs=3) as wpool,
            tc.tile_pool(name="oout", bufs=3) as opool,
        ):
            for t in range(N_TILES):
                xt = xpool.tile([ROWS, TILE_C], F32, tag="xt")
                # in-DMAs on the SP HWDGE ring (qSPDynamicHW)
                nc.sync.dma_start(
                    out=xt[:, :], in_=x[:, t * TILE_C:(t + 1) * TILE_C]
                )
                for h in range(2):
                    xv = xt[:, h * HALF_C:(h + 1) * HALF_C].rearrange(
                        "p (g k) -> p g k", k=B_BITS
                    )
                    a = xv[:, :, 0]
                    b = xv[:, :, 1]
                    c = xv[:, :, 2]
                    d = xv[:, :, 3]

                    u = wpool.tile([ROWS, HALF_G], F32, tag="u")
                    v = wpool.tile([ROWS, HALF_G], F32, tag="v")
                    w = wpool.tile([ROWS, HALF_G], F32, tag="w")
                    ot = opool.tile([ROWS, HALF_G], F32, tag="ot")

                    # u = 2a + b ; v = 2c + d ; w = 4u + v = 8a+4b+2c+d
                    nc.vector.scalar_tensor_tensor(
                        out=u[:, :], in0=a, scalar=2.0, in1=b,
                        op0=MULT, op1=ADD,
                    )
                    nc.vector.scalar_tensor_tensor(
                        out=v[:, :], in0=c, scalar=2.0, in1=d,
                        op0=MULT, op1=ADD,
                    )
                    nc.vector.scalar_tensor_tensor(
                        out=w[:, :], in0=u[:, :], scalar=4.0, in1=v[:, :],
                        op0=MULT, op1=ADD,
                    )
                    # ot = (w + 0.5) / 16 = w/16 + 1/32
                    nc.scalar.activation(
                        out=ot[:, :], in_=w[:, :],
                        func=mybir.ActivationFunctionType.Copy,
                        bias=1.0 / 32.0, scale=1.0 / 16.0,
                    )
                    # out-DMAs on the ACT HWDGE ring (qActDynamicHW) so a
                    # store waiting on compute never blocks the in-stream.
                    nc.scalar.dma_start(
                        out=out[:, t * TILE_G + h * HALF_G:
                                t * TILE_G + (h + 1) * HALF_G],
                        in_=ot[:, :],
                    )
    # Bacc.finalize runs the compile pipeline (register allocation +
    # generate_event_semaphores); the pjrt exec path serializes nc.m as-is.
    nc.finalize()
    return nc


_NC = None


def _get_nc() -> bass.Bass:
    global _NC
    if _NC is None:
        _NC = _build_nc()
    return _NC


def kernel(x: np.ndarray, B=4) -> np.ndarray:
    assert int(B) == B_BITS, f"kernel hardcodes B={B_BITS}, got {B}"
    x = np.ascontiguousarray(x, dtype=np.float32)
    assert x.shape == (BATCH, COLS), x.shape
    nc = _get_nc()
    in_maps = [{"x": x[i * ROWS:(i + 1) * ROWS]} for i in range(N_CORES)]
    res = run_bass_kernel_spmd(nc, in_maps, list(range(N_CORES)))
    return np.concatenate(
        [res.results[i]["out"] for i in range(N_CORES)], axis=0
    )


# revision 10
# speedup vs baseline: 1.2069x; 1.1791x over previous
"""Bit2Num dequantization kernel for Trainium2 (8 NeuronCores, SPMD).

Reference op: x [1024, 65536] of {0.0, 1.0} f32, B=4.
  bits = x.reshape(1024, 16384, 4)
  out[b, n] = (8*bits[b,n,0] + 4*bits[b,n,1] + 2*bits[b,n,2] + bits[b,n,3] + 0.5) / 16

Sharding: pure data-parallel over batch — 128 rows per core (= 128 SBUF
partitions). Per core: 32 MB in + 8 MB out => DMA-roofline-bound (~117 us
at ~358 GB/s HBM-per-NC).

Per-core kernel: pipeline over 8 column tiles of [128, 8192]:
  DMA-in tile -> 3 fused scalar_tensor_tensor ops on DVE (Horner combine of
  the 4 strided bit slices) -> final affine on ACT -> DMA-out [128, 2048].
Compute (~55 us DVE + ~8 us ACT) hides under DMA via tile double/triple
buffering.
"""

import numpy as np

import concourse.bacc as bacc
import concourse.bass as bass
import concourse.mybir as mybir
from concourse.bass_utils import run_bass_kernel_spmd
from concourse.tile import TileContext

N_CORES = 8
BATCH = 1024
COLS = 65536
B_BITS = 4
ROWS = BATCH // N_CORES          # 128 rows per core == SBUF partition count
OUT_COLS = COLS // B_BITS        # 16384
TILE_C = 8192                    # input cols per tile (32 KB / partition)
TILE_G = TILE_C // B_BITS        # 2048 output cols per tile
N_TILES = COLS // TILE_C         # 8

F32 = mybir.dt.float32
MULT = mybir.AluOpType.mult
ADD = mybir.AluOpType.add


def _build_nc() -> bass.Bass:
    # Bacc (not plain Bass): its compile() pipeline runs
    # generate_event_semaphores, which splits multi-wait sync conditions —
    # TRN2 DMA instructions accept at most one wait.
    nc = bacc.Bacc(None, target_bir_lowering=False)
    x = nc.dram_tensor("x", [ROWS, COLS], F32, kind="ExternalInput")
    out = nc.dram_tensor("out", [ROWS, OUT_COLS], F32, kind="ExternalOutput")

    # Compute/store in sub-tile chunks: lets out-DMAs start earlier and
    # shrinks the end-of-kernel exposed tail (last chunk's compute + store).
    with TileContext(nc) as tc:
        with (
            tc.tile_pool(name="xin", bufs=3) as xpool,
            tc.tile_pool(name="work", bufs=3) as wpool,
            tc.tile_pool(name="oout", bufs=3) as opool,
        ):
            for t in range(N_TILES):
                xt = xpool.tile([ROWS, TILE_C], F32, tag="xt")
                # in-DMAs on the SP HWDGE ring (qSPDynamicHW)
                nc.sync.dma_start(
                    out=xt[:, :], in_=x[:, t * TILE_C:(t + 1) * TILE_C]
                )
                # Last tile: finer chunks shrink the exposed tail (the final
                # chunk's compute + store runs after the last in-DMA byte).
                n_chunks = 4 if t == N_TILES - 1 else 2
                chunk_c = TILE_C // n_chunks
                chunk_g = chunk_c // B_BITS
                for h in range(n_chunks):
                    xv = xt[:, h * chunk_c:(h + 1) * chunk_c].rearrange(
                        "p (g k) -> p g k", k=B_BITS
                    )
                    a = xv[:, :, 0]
                    b = xv[:, :, 1]
                    c = xv[:, :, 2]
                    d = xv[:, :, 3]

                    u = wpool.tile([ROWS, chunk_g], F32, tag="u")
                    v = wpool.tile([ROWS, chunk_g], F32, tag="v")
                    w = wpool.tile([ROWS, chunk_g], F32, tag="w")
                    ot = opool.tile([ROWS, chunk_g], F32, tag="ot")

                    # u = 2a + b ; v = 2c + d ; w = 4u + v = 8a+4b+2c+d
                    nc.vector.scalar_tensor_tensor(
                        out=u[:, :], in0=a, scalar=2.0, in1=b,
                        op0=MULT, op1=ADD,
                    )
                    nc.vector.scalar_tensor_tensor(
                        out=v[:, :], in0=c, scalar=2.0, in1=d,
                        op0=MULT, op1=ADD,
                    )
                    nc.vector.scalar_tensor_tensor(
                        out=w[:, :], in0=u[:, :], scalar=4.0, in1=v[:, :],
                        op0=MULT, op1=ADD,
                    )
                    # ot = (w + 0.5) / 16 = w/16 + 1/32
                    nc.scalar.activation(
                        out=ot[:, :], in_=w[:, :],
                        func=mybir.ActivationFunctionType.Copy,
                        bias=1.0 / 32.0, scale=1.0 / 16.0,
                    )
                    # out-DMAs on the ACT HWDGE ring (qActDynamicHW) so a
                    # store waiting on compute never blocks the in-stream.
                    nc.scalar.dma_start(
                        out=out[:, t * TILE_G + h * chunk_g:
                                t * TILE_G + (h + 1) * chunk_g],
                        in_=ot[:, :],
                    )
    # Bacc.finalize runs the compile pipeline (register allocation +
    # generate_event_semaphores); the pjrt exec path serializes nc.m as-is.
    nc.finalize()
    return nc


_NC = None


def _get_nc() -> bass.Bass:
    global _NC
    if _NC is None:
        _NC = _build_nc()
    return _NC


def kernel(x: np.ndarray, B=4) -> np.ndarray:
    assert int(B) == B_BITS, f"kernel hardcodes B={B_BITS}, got {B}"
    x = np.ascontiguousarray(x, dtype=np.float32)
    assert x.shape == (BATCH, COLS), x.shape
    nc = _get_nc()
    in_maps = [{"x": x[i * ROWS:(i + 1) * ROWS]} for i in range(N_CORES)]
    res = run_bass_kernel_spmd(nc, in_maps, list(range(N_CORES)))
    return np.concatenate(
        [res.results[i]["out"] for i in range(N_CORES)], axis=0
    )
